# revision 26
# baseline (speedup 1.0000x reference)
"""Trainium2 Bass kernel for nn_InteractionLayer (gnn_message_passing).

Computes out = S @ (z @ W + B) where
  S[i,j] = exp(-(1/d[i,j] - 1/mu)^2 / (2 sigma^2)) * [d[i,j] < 0.5] * [i != j]

Strategy (8-way row-parallel over destination nodes i, no collectives):
  - Each core owns a 1024-row slab of dist and the matching output rows.
  - out = S @ h contracts j, so S must sit in SBUF with j on partitions:
    every 128x128 block of the dist slab is transposed on the PE (fp32
    transpose mode, bit-exact).
  - The Gaussian argument x(d) = (1/d - 1/mu)/(sqrt2 sigma) only matters on
    the narrow support where S > ~1e-33 (d in ~[0.48, 0.5) for this mu/sigma);
    everywhere else it just has to stay past the LUT's underflow point. So
    1/d is replaced by a parabola: x(d) ~ e*(d-f)^2 + g, fitted at build time
    from the actual mu/sigma (max fit error ~1e-4, verified dead outside).
    This removes the reciprocal (the DVE reciprocal runs at ~1/6 line rate).
  - The cutoff mask (exact is_ge on untouched fp32 d) and the -f recentering
    ride one DVE tensor_scalar: m = (d >= 0.5) + f in {f, 1+f} (fp8e4-exact
    by construction), transposed-and-subtracted into the same PSUM tile as
    the d transpose via a normal fp8 matmul against -Identity. Masked-out
    entries land at t = d-1-f, whose parabola value is >> dead threshold.
  - y = t^2 on DVE (tensor_tensor) and ACT (Square), split to balance load;
    one ScalarE Derivative_Erf pass evaluates 2/sqrt(pi)*exp(-(e*y+g)^2);
    the 2/sqrt(pi) is folded into W/B on the host.
  - Big matmul in fp32r (tf32-like) accumulating out.T in PSUM:
    out.T[f_, i] = sum_j h[j, f_] * S.T[j, i], N=512 so fp32r is full rate.
  - h = z @ W + B is computed on-device per core from a host-transposed z.T.
The host only slices/replicates inputs, kills the diagonal in each shard
(those elements are structurally masked out), and transposes the returned
out.T slabs.
"""

import math
import numpy as np

N, D_IN, D_OUT = 8192, 128, 128
NCORES = 8
ROWS = N // NCORES          # 1024 destination rows per core
P = 128                     # partitions
JWIN = 1024                 # dist column window staged in SBUF
NPANEL = ROWS // P          # 8 i-panels per core
NWIN = N // JWIN            # 8 column windows
NJB = N // P                # 64 j-blocks total
DEAD = np.float32(0.75)     # provably-masked value used for the diagonal
DVE_SQUARE_MOD = 4          # of every 4 j-blocks, 3 square on DVE, 1 on ACT

_CACHE = {}


def _fit_parabola(mu_v: float, sg_v: float):
    """Fit x(d) = (1/d - 1/mu)/(sqrt2 sigma) ~ e*(d-f)^2 + g on the S-support.

    f is constrained so the mask/recenter constants {f, 1+f} survive the
    fp8e4 matmul path exactly. Returns (f, e, g) and
    asserts the approximation is accurate on the support and dead outside.
    """
    c = 1.0 / mu_v
    s2 = 1.0 / (math.sqrt(2.0) * sg_v)

    def xfun(d):
        return (1.0 / d - c) * s2

    # accuracy-critical: S >= 1e-33  <=>  |x| <= ~8.7 ; solve for dlo
    lo, hi = 1e-6, 0.5
    if xfun(hi - 1e-9) > 8.8:   # whole support already dead => any parabola
        dlo = hi - 0.02
    else:
        for _ in range(80):
            mid = 0.5 * (lo + hi)
            if xfun(mid) > 8.8:
                lo = mid
            else:
                hi = mid
        dlo = lo - 0.002
    ds = np.linspace(dlo, 0.5000002, 4001)
    xs = xfun(ds)
    # Weight each sample by its impact on the output: d(S)/S = 2x * dx and S
    # itself spans many decades, so weight ~ x * exp(-(x^2 - xmin^2)) (clipped)
    # concentrates accuracy where S actually contributes.
    xmin = xs.min()
    wgt = xs * np.exp(np.clip(xmin ** 2 - xs ** 2, -30.0, 0.0)) + 1e-6
    best = None

    # f and 1+f ride an fp8e4m3 matmul operand, so both must be exactly
    # representable there: f on the eighths grid guarantees it.
    for f in (0.625, 0.75, 0.875):
        A = np.stack([(ds - f) ** 2, np.ones_like(ds)], 1)
        coef, *_ = np.linalg.lstsq(A * wgt[:, None], xs * wgt, rcond=None)
        werr = (np.abs(A @ coef - xs) * wgt).max()
        if best is None or werr < best[0]:
            best = (werr, f, float(coef[0]), float(coef[1]))
    werr, f, e, g = best
    # a few reweighted iterations toward weighted-minimax
    A = np.stack([(ds - f) ** 2, np.ones_like(ds)], 1)
    coef = np.array([e, g])
    for _ in range(60):
        r = np.abs(A @ coef - xs) * wgt
        w2 = wgt * (r + 1e-12) ** 0.5
        coef = 0.5 * (coef + np.linalg.lstsq(A * w2[:, None], xs * w2, rcond=None)[0])
    e, g = float(coef[0]), float(coef[1])
    err = np.abs(A @ coef - xs).max()
    assert err < 5e-3, f"parabola fit too coarse: {err}"
    # dead-zone checks: masked-in below support, and masked-out (t = d-1-f)
    dd = np.linspace(1e-6, dlo, 2001)
    assert np.all(e * (dd - f) ** 2 + g > 8.9), "parabola not dead below support"
    dd = np.linspace(0.5, 1.0, 2001) - 1.0
    assert np.all(e * (dd - f) ** 2 + g > 8.9), "parabola not dead for masked-out"
    assert e * (DEAD - 1.0 - f) ** 2 + g > 8.9, "diagonal fill not dead"
    return f, e, g


def _build_program(f: float, e: float, g: float):
    from contextlib import ExitStack
    import concourse.tile as tile
    from concourse import bacc, mybir
    from concourse.masks import make_identity

    nc = bacc.Bacc(num_devices=NCORES)
    d_d = nc.dram_tensor("dsh", [ROWS, N], mybir.dt.float32, kind="ExternalInput")
    zT_d = nc.dram_tensor("zT", [D_IN, N], mybir.dt.float32, kind="ExternalInput")
    w_d = nc.dram_tensor("w", [D_IN, D_OUT], mybir.dt.float32, kind="ExternalInput")
    b_d = nc.dram_tensor("b", [1, D_OUT], mybir.dt.float32, kind="ExternalInput")
    o_d = nc.dram_tensor("outT", [D_OUT, ROWS], mybir.dt.float32, kind="ExternalOutput")

    f32, f32r, bf16 = mybir.dt.float32, mybir.dt.float32r, mybir.dt.bfloat16
    A = mybir.AluOpType
    AF = mybir.ActivationFunctionType

    with tile.TileContext(nc) as tc, ExitStack() as ctx:
        consts = ctx.enter_context(tc.tile_pool(name="consts", bufs=1))
        hpool = ctx.enter_context(tc.tile_pool(name="h", bufs=1))
        outps = ctx.enter_context(tc.tile_pool(name="outps", bufs=1, space="PSUM"))

        ident = consts.tile([P, P], f32)
        make_identity(nc, ident)
        f8 = mybir.dt.float8e4
        nident_f8 = consts.tile([P, P], f8)
        nc.vector.tensor_scalar(nident_f8, ident, -1.0, None, A.mult)
        bias_g = consts.tile([P, 1], f32)
        nc.vector.memset(bias_g, float(g))
        w_r = consts.tile([D_IN, D_OUT], f32r)
        nc.scalar.dma_start(w_r, w_d.ap().bitcast(f32r))
        import concourse.bass as bass_mod
        b_bc = consts.tile([P, 4 * D_OUT], f32)   # B broadcast to all partitions, x4
        for q in range(4):
            bav = b_d.ap()
            b_bcast_ap = bass_mod.AP(
                tensor=bav.tensor, offset=bav.offset,
                ap=[[0, P]] + bav.ap[1:])
            nc.scalar.dma_start(b_bc[:, q * D_OUT:(q + 1) * D_OUT], b_bcast_ap)

        h_sb = hpool.tile([P, N], f32r)          # h[j, f_] as 64 [128,128] column slices
        outT = outps.tile([D_OUT, ROWS], f32)    # accumulates over all 64 j-blocks

        # ---- phase A: h = z @ W + B (per-core replica) ----
        # h matmuls are emitted per zT chunk so the PE has work within ~2.5us
        # of kernel start instead of waiting for the whole z.T to land.
        with tc.tile_pool(name="phA", bufs=1) as pha, \
             tc.tile_pool(name="phAps", bufs=2, space="PSUM") as phaps:
            zT_sb = pha.tile([D_IN, N], f32r)
            for i in range(NWIN):
                nc.scalar.dma_start(zT_sb[:, i * JWIN:(i + 1) * JWIN],
                                  zT_d.ap()[:, i * JWIN:(i + 1) * JWIN].bitcast(f32r))
                for grp in range(2 * i, 2 * i + 2):
                    hps = phaps.tile([P, 4 * P], f32)
                    for q in range(4):
                        jb = grp * 4 + q
                        sl = slice(q * P, (q + 1) * P)
                        nc.tensor.matmul(hps[:, sl], zT_sb[:, jb * P:(jb + 1) * P],
                                         w_r, start=True, stop=True)
                    # h = zW + B: the eviction adds the broadcast B on the DVE
                    nc.vector.tensor_tensor(h_sb[:, grp * 4 * P:(grp + 1) * 4 * P],
                                            hps, b_bc, A.add)

        # ---- phase B: mask+recenter, transpose, square, Gaussian, matmul ----
        dnat = ctx.enter_context(tc.tile_pool(name="dnat", bufs=2 * NPANEL))
        mpool = ctx.enter_context(tc.tile_pool(name="m", bufs=2 * NPANEL))
        tpool = ctx.enter_context(tc.tile_pool(name="tsb", bufs=2))
        ypool = ctx.enter_context(tc.tile_pool(name="y", bufs=3))
        spool = ctx.enter_context(tc.tile_pool(name="st", bufs=3))
        dpt_ps = ctx.enter_context(tc.tile_pool(name="dpt", bufs=3, space="PSUM"))

        def emit_transpose_pair(dts, mts, jb):
            # Both j-blocks of a pair: all 16 fp32 transposes, then all 16
            # fp8 mask matmuls -- long same-dtype weight-load runs keep FWL
            # alive and the PE weight pipeline dense.
            dpts = []
            for jbx in (jb, jb + 1):
                sl_j = slice(jbx * P, (jbx + 1) * P)
                dpt = dpt_ps.tile([P, ROWS], f32, tag="dpt")
                for p in range(NPANEL):
                    sl_i = slice(p * P, (p + 1) * P)
                    nc.tensor.matmul(dpt[:, sl_i], dts[p][:, sl_j], ident,
                                     is_transpose=True, start=(p % 4 == 0),
                                     stop=False, skip_group_check=True)
                dpts.append(dpt)
            for k, jbx in enumerate((jb, jb + 1)):
                sl_j = slice(jbx * P, (jbx + 1) * P)
                for p in range(NPANEL):
                    sl_i = slice(p * P, (p + 1) * P)
                    nc.tensor.matmul(dpts[k][:, sl_i], mts[p][:, sl_j], nident_f8,
                                     start=False, stop=True, skip_group_check=True)
            return dpts

        for w in range(NWIN):
            dts = []
            mts = []
            for p in range(NPANEL):
                dt_ = dnat.tile([P, JWIN], f32, tag="dnat")
                nc.sync.dma_start(
                    dt_, d_d.ap()[p * P:(p + 1) * P, w * JWIN:(w + 1) * JWIN])
                # m = (d >= 0.5) + f  in {f, 1+f}; both ends bf16-exact
                mt = mpool.tile([P, JWIN], f8, tag="m")
                nc.vector.tensor_scalar(mt, dt_, 0.5, float(f), A.is_ge, A.add)
                dts.append(dt_)
                mts.append(mt)
            for jb in range(0, JWIN // P, 2):
                jg = w * (JWIN // P) + jb
                dptA, dptB = emit_transpose_pair(dts, mts, jb)
                y = ypool.tile([P, 2 * ROWS], f32, tag="y")
                # square: ACT takes most tiles; DVE (copy + mult) takes the rest
                if (jg // 2) % 4 == 3:
                    nc.scalar.activation(y[:, :ROWS], dptA, AF.Square)
                else:
                    t_sb = tpool.tile([P, ROWS], f32, tag="tsb")
                    nc.vector.tensor_copy(t_sb, dptA)
                    nc.vector.tensor_tensor(y[:, :ROWS], t_sb, t_sb, A.mult)
                nc.scalar.activation(y[:, ROWS:], dptB, AF.Square)
                st = spool.tile([P, 2 * ROWS], f32r, tag="st")
                nc.scalar.activation(st, y, AF.Derivative_Erf,
                                     bias=bias_g, scale=float(e))
                for k, jb2 in enumerate((jg, jg + 1)):
                    for half in range(2):
                        io = slice(half * 512, (half + 1) * 512)
                        so = slice(k * ROWS + half * 512, k * ROWS + (half + 1) * 512)
                        nc.tensor.matmul(outT[:, io], h_sb[:, jb2 * P:(jb2 + 1) * P],
                                         st[:, so],
                                         start=(jb2 == 0), stop=(jb2 == NJB - 1))

        # ---- phase C: store out.T ----
        osb = ctx.enter_context(tc.tile_pool(name="osb", bufs=1))
        ot = osb.tile([D_OUT, ROWS], f32)
        nc.scalar.activation(ot, outT, AF.Copy)
        nc.sync.dma_start(o_d.ap(), ot)

    nc.compile()
    return nc


def kernel(z, dist_matrix, W, B, mu, sigma):
    from concourse.bass_utils import run_bass_kernel_spmd

    z = np.asarray(z, dtype=np.float32)
    dist_matrix = np.asarray(dist_matrix, dtype=np.float32)
    W = np.asarray(W, dtype=np.float32)
    B = np.asarray(B, dtype=np.float32)
    mu_v = float(np.asarray(mu).reshape(-1)[0])
    sg_v = float(np.asarray(sigma).reshape(-1)[0])

    f, e, g = _fit_parabola(mu_v, abs(sg_v))
    # DErf(e*y + g) = 2/sqrt(pi) exp(-x^2): fold sqrt(pi)/2 into W and B.
    fold = math.sqrt(math.pi) / 2.0
    Wf = (W.astype(np.float64) * fold).astype(np.float32)
    Bf = (B.astype(np.float64) * fold).astype(np.float32).reshape(1, D_OUT)

    key = (f, e, g)
    if key not in _CACHE:
        _CACHE[key] = _build_program(f, e, g)
    nc = _CACHE[key]

    zT = np.ascontiguousarray(z.T)
    in_maps = []
    for c in range(NCORES):
        dsh = dist_matrix[c * ROWS:(c + 1) * ROWS].copy()
        rr = np.arange(ROWS)
        dsh[rr, c * ROWS + rr] = DEAD   # i == j is structurally masked out
        in_maps.append({"dsh": dsh, "zT": zT, "w": Wf, "b": Bf})

    res = run_bass_kernel_spmd(nc, in_maps, core_ids=list(range(NCORES)))
    global LAST_RESULTS, LAST_EXEC_NS
    LAST_RESULTS = res
    LAST_EXEC_NS = res.exec_time_ns
    out = np.empty((N, D_OUT), dtype=np.float32)
    for c in range(NCORES):
        out[c * ROWS:(c + 1) * ROWS] = res.results[c]["outT"].T
    return out


LAST_RESULTS = None
LAST_EXEC_NS = None


# revision 27
# speedup vs baseline: 1.0143x; 1.0143x over previous
"""Trainium2 Bass kernel for nn_InteractionLayer (gnn_message_passing).

Computes out = S @ (z @ W + B) where
  S[i,j] = exp(-(1/d[i,j] - 1/mu)^2 / (2 sigma^2)) * [d[i,j] < 0.5] * [i != j]

Strategy (8-way row-parallel over destination nodes i, no collectives):
  - Each core owns a 1024-row slab of dist and the matching output rows.
  - out = S @ h contracts j, so S must sit in SBUF with j on partitions:
    every 128x128 block of the dist slab is transposed on the PE (fp32
    transpose mode, bit-exact).
  - The Gaussian argument x(d) = (1/d - 1/mu)/(sqrt2 sigma) only matters on
    the narrow support where S > ~1e-33 (d in ~[0.48, 0.5) for this mu/sigma);
    everywhere else it just has to stay past the LUT's underflow point. So
    1/d is replaced by a parabola: x(d) ~ e*(d-f)^2 + g, fitted at build time
    from the actual mu/sigma (max fit error ~1e-4, verified dead outside).
    This removes the reciprocal (the DVE reciprocal runs at ~1/6 line rate).
  - The cutoff mask (exact is_ge on untouched fp32 d) and the -f recentering
    ride one DVE tensor_scalar: m = (d >= 0.5) + f in {f, 1+f} (fp8e4-exact
    by construction), transposed-and-subtracted into the same PSUM tile as
    the d transpose via a normal fp8 matmul against -Identity. Masked-out
    entries land at t = d-1-f, whose parabola value is >> dead threshold.
  - y = t^2 on DVE (tensor_tensor) and ACT (Square), split to balance load;
    one ScalarE Derivative_Erf pass evaluates 2/sqrt(pi)*exp(-(e*y+g)^2);
    the 2/sqrt(pi) is folded into W/B on the host.
  - Big matmul in fp32r (tf32-like) accumulating out.T in PSUM:
    out.T[f_, i] = sum_j h[j, f_] * S.T[j, i], N=512 so fp32r is full rate.
  - h = z @ W + B is computed on-device per core from a host-transposed z.T.
The host only slices/replicates inputs, kills the diagonal in each shard
(those elements are structurally masked out), and transposes the returned
out.T slabs.
"""

import math
import numpy as np

N, D_IN, D_OUT = 8192, 128, 128
NCORES = 8
ROWS = N // NCORES          # 1024 destination rows per core
P = 128                     # partitions
JWIN = 1024                 # dist column window staged in SBUF
NPANEL = ROWS // P          # 8 i-panels per core
NWIN = N // JWIN            # 8 column windows
NJB = N // P                # 64 j-blocks total
DEAD = np.float32(0.75)     # provably-masked value used for the diagonal

_CACHE = {}


def _fit_parabola(mu_v: float, sg_v: float):
    """Fit x(d) = (1/d - 1/mu)/(sqrt2 sigma) ~ e*(d-f)^2 + g on the S-support.

    f is constrained so the mask/recenter constants {f, 1+f} survive the
    fp8e4 matmul path exactly. Returns (f, e, g) and
    asserts the approximation is accurate on the support and dead outside.
    """
    c = 1.0 / mu_v
    s2 = 1.0 / (math.sqrt(2.0) * sg_v)

    def xfun(d):
        return (1.0 / d - c) * s2

    # accuracy-critical: S >= 1e-33  <=>  |x| <= ~8.7 ; solve for dlo
    lo, hi = 1e-6, 0.5
    if xfun(hi - 1e-9) > 8.8:   # whole support already dead => any parabola
        dlo = hi - 0.02
    else:
        for _ in range(80):
            mid = 0.5 * (lo + hi)
            if xfun(mid) > 8.8:
                lo = mid
            else:
                hi = mid
        dlo = lo - 0.002
    ds = np.linspace(dlo, 0.5000002, 4001)
    xs = xfun(ds)
    # Weight each sample by its impact on the output: d(S)/S = 2x * dx and S
    # itself spans many decades, so weight ~ x * exp(-(x^2 - xmin^2)) (clipped)
    # concentrates accuracy where S actually contributes.
    xmin = xs.min()
    wgt = xs * np.exp(np.clip(xmin ** 2 - xs ** 2, -30.0, 0.0)) + 1e-6
    best = None

    # f and 1+f ride an fp8e4m3 matmul operand, so both must be exactly
    # representable there: f on the eighths grid guarantees it.
    for f in (0.625, 0.75, 0.875):
        A = np.stack([(ds - f) ** 2, np.ones_like(ds)], 1)
        coef, *_ = np.linalg.lstsq(A * wgt[:, None], xs * wgt, rcond=None)
        werr = (np.abs(A @ coef - xs) * wgt).max()
        if best is None or werr < best[0]:
            best = (werr, f, float(coef[0]), float(coef[1]))
    werr, f, e, g = best
    # a few reweighted iterations toward weighted-minimax
    A = np.stack([(ds - f) ** 2, np.ones_like(ds)], 1)
    coef = np.array([e, g])
    for _ in range(60):
        r = np.abs(A @ coef - xs) * wgt
        w2 = wgt * (r + 1e-12) ** 0.5
        coef = 0.5 * (coef + np.linalg.lstsq(A * w2[:, None], xs * w2, rcond=None)[0])
    e, g = float(coef[0]), float(coef[1])
    err = np.abs(A @ coef - xs).max()
    assert err < 5e-3, f"parabola fit too coarse: {err}"
    # dead-zone checks: masked-in below support, and masked-out (t = d-1-f)
    dd = np.linspace(1e-6, dlo, 2001)
    assert np.all(e * (dd - f) ** 2 + g > 8.9), "parabola not dead below support"
    dd = np.linspace(0.5, 1.0, 2001) - 1.0
    assert np.all(e * (dd - f) ** 2 + g > 8.9), "parabola not dead for masked-out"
    assert e * (DEAD - 1.0 - f) ** 2 + g > 8.9, "diagonal fill not dead"
    return f, e, g


def _build_program(f: float, e: float, g: float):
    from contextlib import ExitStack
    import concourse.tile as tile
    from concourse import bacc, mybir
    from concourse.masks import make_identity

    nc = bacc.Bacc(num_devices=NCORES)
    d_d = nc.dram_tensor("dsh", [ROWS, N], mybir.dt.float32, kind="ExternalInput")
    zT_d = nc.dram_tensor("zT", [D_IN, N], mybir.dt.float32, kind="ExternalInput")
    w_d = nc.dram_tensor("w", [D_IN, D_OUT], mybir.dt.float32, kind="ExternalInput")
    b_d = nc.dram_tensor("b", [1, D_OUT], mybir.dt.float32, kind="ExternalInput")
    o_d = nc.dram_tensor("outT", [D_OUT, ROWS], mybir.dt.float32, kind="ExternalOutput")

    f32, f32r = mybir.dt.float32, mybir.dt.float32r
    A = mybir.AluOpType
    AF = mybir.ActivationFunctionType

    with tile.TileContext(nc) as tc, ExitStack() as ctx:
        consts = ctx.enter_context(tc.tile_pool(name="consts", bufs=1))
        hpool = ctx.enter_context(tc.tile_pool(name="h", bufs=1))
        outps = ctx.enter_context(tc.tile_pool(name="outps", bufs=1, space="PSUM"))

        ident = consts.tile([P, P], f32)
        make_identity(nc, ident)
        f8 = mybir.dt.float8e4
        nident_f8 = consts.tile([P, P], f8)
        nc.vector.tensor_scalar(nident_f8, ident, -1.0, None, A.mult)
        bias_g = consts.tile([P, 1], f32)
        nc.vector.memset(bias_g, float(g))
        w_r = consts.tile([D_IN, D_OUT], f32r)
        nc.scalar.dma_start(w_r, w_d.ap().bitcast(f32r))
        import concourse.bass as bass_mod
        b_bc = consts.tile([P, 4 * D_OUT], f32)   # B broadcast to all partitions, x4
        for q in range(4):
            bav = b_d.ap()
            b_bcast_ap = bass_mod.AP(
                tensor=bav.tensor, offset=bav.offset,
                ap=[[0, P]] + bav.ap[1:])
            nc.scalar.dma_start(b_bc[:, q * D_OUT:(q + 1) * D_OUT], b_bcast_ap)

        h_sb = hpool.tile([P, N], f32r)          # h[j, f_] as 64 [128,128] column slices
        outT = outps.tile([D_OUT, ROWS], f32)    # accumulates over all 64 j-blocks

        # ---- phase A: h = z @ W + B (per-core replica) ----
        # h matmuls are emitted per zT chunk so the PE has work within ~2.5us
        # of kernel start instead of waiting for the whole z.T to land.
        with tc.tile_pool(name="phA", bufs=1) as pha, \
             tc.tile_pool(name="phAps", bufs=2, space="PSUM") as phaps:
            zT_sb = pha.tile([D_IN, N], f32r)
            for i in range(NWIN):
                nc.scalar.dma_start(zT_sb[:, i * JWIN:(i + 1) * JWIN],
                                  zT_d.ap()[:, i * JWIN:(i + 1) * JWIN].bitcast(f32r))
                for grp in range(2 * i, 2 * i + 2):
                    hps = phaps.tile([P, 4 * P], f32)
                    for q in range(4):
                        jb = grp * 4 + q
                        sl = slice(q * P, (q + 1) * P)
                        nc.tensor.matmul(hps[:, sl], zT_sb[:, jb * P:(jb + 1) * P],
                                         w_r, start=True, stop=True)
                    # h = zW + B: the eviction adds the broadcast B on the DVE
                    nc.vector.tensor_tensor(h_sb[:, grp * 4 * P:(grp + 1) * 4 * P],
                                            hps, b_bc, A.add)

        # ---- phase B: mask+recenter, transpose, square, Gaussian, matmul ----
        dnat = ctx.enter_context(tc.tile_pool(name="dnat", bufs=2 * NPANEL))
        mpool = ctx.enter_context(tc.tile_pool(name="m", bufs=2 * NPANEL))
        tpool = ctx.enter_context(tc.tile_pool(name="tsb", bufs=2))
        ypool = ctx.enter_context(tc.tile_pool(name="y", bufs=3))
        spool = ctx.enter_context(tc.tile_pool(name="st", bufs=3))
        dpt_ps = ctx.enter_context(tc.tile_pool(name="dpt", bufs=3, space="PSUM"))

        def emit_transpose_pair(dts, mts, jb):
            # Both j-blocks of a pair: all 16 fp32 transposes, then all 16
            # fp8 mask matmuls -- long same-dtype weight-load runs keep FWL
            # alive and the PE weight pipeline dense.
            dpts = []
            for jbx in (jb, jb + 1):
                sl_j = slice(jbx * P, (jbx + 1) * P)
                dpt = dpt_ps.tile([P, ROWS], f32, tag="dpt")
                for p in range(NPANEL):
                    sl_i = slice(p * P, (p + 1) * P)
                    nc.tensor.matmul(dpt[:, sl_i], dts[p][:, sl_j], ident,
                                     is_transpose=True, start=(p % 4 == 0),
                                     stop=False, skip_group_check=True)
                dpts.append(dpt)
            for k, jbx in enumerate((jb, jb + 1)):
                sl_j = slice(jbx * P, (jbx + 1) * P)
                for p in range(NPANEL):
                    sl_i = slice(p * P, (p + 1) * P)
                    nc.tensor.matmul(dpts[k][:, sl_i], mts[p][:, sl_j], nident_f8,
                                     start=False, stop=True, skip_group_check=True)
            return dpts

        for w in range(NWIN):
            dts = []
            mts = []
            for p in range(NPANEL):
                dt_ = dnat.tile([P, JWIN], f32, tag="dnat")
                nc.sync.dma_start(
                    dt_, d_d.ap()[p * P:(p + 1) * P, w * JWIN:(w + 1) * JWIN])
                # m = (d >= 0.5) + f  in {f, 1+f}; both ends fp8e4-exact
                mt = mpool.tile([P, JWIN], f8, tag="m")
                nc.vector.tensor_scalar(mt, dt_, 0.5, float(f), A.is_ge, A.add)
                dts.append(dt_)
                mts.append(mt)
            for jb in range(0, JWIN // P, 2):
                jg = w * (JWIN // P) + jb
                dptA, dptB = emit_transpose_pair(dts, mts, jb)
                y = ypool.tile([P, 2 * ROWS], f32, tag="y")
                # square: ACT takes most tiles; DVE (copy + mult) takes the rest
                if (jg // 2) % 4 == 3:
                    nc.scalar.activation(y[:, :ROWS], dptA, AF.Square)
                else:
                    t_sb = tpool.tile([P, ROWS], f32, tag="tsb")
                    nc.vector.tensor_copy(t_sb, dptA)
                    nc.vector.tensor_tensor(y[:, :ROWS], t_sb, t_sb, A.mult)
                nc.scalar.activation(y[:, ROWS:], dptB, AF.Square)
                st = spool.tile([P, 2 * ROWS], f32r, tag="st")
                nc.scalar.activation(st, y, AF.Derivative_Erf,
                                     bias=bias_g, scale=float(e))
                for k, jb2 in enumerate((jg, jg + 1)):
                    for half in range(2):
                        io = slice(half * 512, (half + 1) * 512)
                        so = slice(k * ROWS + half * 512, k * ROWS + (half + 1) * 512)
                        nc.tensor.matmul(outT[:, io], h_sb[:, jb2 * P:(jb2 + 1) * P],
                                         st[:, so],
                                         start=(jb2 == 0), stop=(jb2 == NJB - 1))

        # ---- phase C: store out.T ----
        osb = ctx.enter_context(tc.tile_pool(name="osb", bufs=1))
        ot = osb.tile([D_OUT, ROWS], f32)
        nc.scalar.activation(ot, outT, AF.Copy)
        nc.sync.dma_start(o_d.ap(), ot)

    nc.compile()
    return nc


def kernel(z, dist_matrix, W, B, mu, sigma):
    from concourse.bass_utils import run_bass_kernel_spmd

    z = np.asarray(z, dtype=np.float32)
    dist_matrix = np.asarray(dist_matrix, dtype=np.float32)
    W = np.asarray(W, dtype=np.float32)
    B = np.asarray(B, dtype=np.float32)
    mu_v = float(np.asarray(mu).reshape(-1)[0])
    sg_v = float(np.asarray(sigma).reshape(-1)[0])

    f, e, g = _fit_parabola(mu_v, abs(sg_v))
    # DErf(e*y + g) = 2/sqrt(pi) exp(-x^2): fold sqrt(pi)/2 into W and B.
    fold = math.sqrt(math.pi) / 2.0
    Wf = (W.astype(np.float64) * fold).astype(np.float32)
    Bf = (B.astype(np.float64) * fold).astype(np.float32).reshape(1, D_OUT)

    key = (f, e, g)
    if key not in _CACHE:
        _CACHE[key] = _build_program(f, e, g)
    nc = _CACHE[key]

    zT = np.ascontiguousarray(z.T)
    in_maps = []
    for c in range(NCORES):
        dsh = dist_matrix[c * ROWS:(c + 1) * ROWS].copy()
        rr = np.arange(ROWS)
        dsh[rr, c * ROWS + rr] = DEAD   # i == j is structurally masked out
        in_maps.append({"dsh": dsh, "zT": zT, "w": Wf, "b": Bf})

    res = run_bass_kernel_spmd(nc, in_maps, core_ids=list(range(NCORES)))
    global LAST_RESULTS, LAST_EXEC_NS
    LAST_RESULTS = res
    LAST_EXEC_NS = res.exec_time_ns
    out = np.empty((N, D_OUT), dtype=np.float32)
    for c in range(NCORES):
        out[c * ROWS:(c + 1) * ROWS] = res.results[c]["outT"].T
    return out


LAST_RESULTS = None
LAST_EXEC_NS = None
